# revision 39
# baseline (speedup 1.0000x reference)
# Trainium2 Bass kernel for nn_CompressedGPT2Attention.
#
# Model: B=2, S=2048, D=1024, H=16 heads of HD=64.
#   qkv = x @ c_attn_w + c_attn_b ; causal attention per head;
#   per-head symmetric projector on the attention output; out = attn @ c_proj_w + b.
#
# Sharding (megatron-style tensor parallel over heads, 8 cores x 2 heads):
#   - every core gets the full hidden_states
#   - c_attn (q,k,v) columns sharded by head; the per-head projector is folded
#     into c_proj on the host (W~_h = proj_h @ c_proj_rows_h)
#   - each core writes a full-shape bf16 partial; host does the f32 all-reduce
#     + bias.
#
# v4 changes (vs the 190us v3):
#   - softmax reciprocal via the single-instruction DVE reciprocal_approx_fast
#     (reads the sums PSUM bank directly) instead of the 3.3us iterative
#     RECIPROCAL / the ScalarE ln+exp trick. ScalarE now only ever runs
#     Exp/Identity, which share one ACT table set -> no mid-kernel
#     ACT_TABLE_LOADs (was 9us of them).
#   - causal mask applied by accumulating a constant -240 strict-lower
#     triangular matrix into the scores PSUM with one extra PE matmul
#     (128-col identity stream) instead of gpsimd affine_select on e after
#     exp: removes the exp->gpsimd->attn cross-engine hop on every diagonal
#     step, and exp(score-240*mask) underflows to 0 on its own.
#   - startup weight DMAs and the first x chunk are split per 128-row k-tile
#     so the first qkv matmul only waits for the first 64-128KB slice, not the
#     full tensor (later chunks stay single-trigger: SWDGE descriptor gen is
#     ~630ns per 2D dma_start on the gpsimd queue).
#   - ~30 junk 128-col matmuls on const tiles warm the PE HAM clock gate
#     (4/8 -> 8/8 takes ~3.4us of sustained busy) while the startup DMAs are
#     still in flight, so qkv starts at 2.4GHz instead of 1.2.
#   - the final block's output DMAs alternate between the sync and gpsimd
#     queues so the 1MB tail drain uses two rings.

import numpy as np

B, S, D, H, HD = 2, 2048, 1024, 16, 64
BS = B * S
N_CORES = 8
HPC = H // N_CORES  # heads per core = 2

_CACHE = {}


def _build(nc):
    import concourse.bass as bass
    import concourse.mybir as mybir
    import concourse.tile as tile
    from contextlib import ExitStack

    f32 = mybir.dt.float32
    bf16 = mybir.dt.bfloat16
    AF = mybir.ActivationFunctionType
    OP = mybir.AluOpType

    x_d = nc.dram_tensor("xT", [D, BS], bf16, kind="ExternalInput").ap()
    wqk_d = nc.dram_tensor("w_qk", [D, 2 * HPC * HD], bf16, kind="ExternalInput").ap()
    wv_d = nc.dram_tensor("w_v", [D, HPC * HD], bf16, kind="ExternalInput").ap()
    bqk_d = nc.dram_tensor("b_qk", [2 * HPC * HD], f32, kind="ExternalInput").ap()
    bv_d = nc.dram_tensor("b_v512", [512], bf16, kind="ExternalInput").ap()
    wtil_d = nc.dram_tensor("w_til", [HPC * HD, D], bf16, kind="ExternalInput").ap()
    out_d = nc.dram_tensor("outT", [8, 128, BS], bf16, kind="ExternalOutput").ap()

    F = HPC * HD  # 128 features (2 heads stacked)
    NB = BS // 512
    KT = D // 128

    with TileCtx(tile, nc) as tc:
        frees = []

        def ptile(shape, dtype, name):
            t, free = tc.tile(shape, dtype, name=name)
            frees.append(free)
            return t

        qT = ptile([128, BS], bf16, "qT")
        kTt = ptile([128, BS], bf16, "kTt")
        v_s = ptile([128, BS // 128, 128], bf16, "v_s")
        wqk_sb = ptile([128, KT, 2 * F], bf16, "wqk_sb")
        wv_sb = ptile([128, KT, F], bf16, "wv_sb")
        wtil_sb = ptile([128, D], bf16, "wtil_sb")
        bqk_sb = ptile([128, 2], f32, "bqk_sb")
        ones_w = ptile([128, 64], bf16, "ones_w")
        ones_row = ptile([1, 128], bf16, "ones_row")
        bv16 = ptile([1, 512], bf16, "bv16")
        bias_v_bc = ptile([128, 512], f32, "bias_v_bc")
        dummy = ptile([1, 2], f32, "dummy")
        wm = ptile([128, 128], bf16, "wm")  # causal-mask weight: -240 where k>j
        ident = ptile([128, 128], bf16, "ident")
        cpr = [ptile([128, 512], bf16, f"cpr{i}") for i in range(NB)]

        nc.vector.memset(ones_w[:], 1.0)
        nc.vector.memset(ones_row[:], 1.0)
        nc.vector.memset(dummy[:], 1.0)
        # preload the exp ACT table set while the PE ramps on qkv; Identity
        # shares the same set so this is the only table load in the kernel
        nc.scalar.activation(dummy[:, 0:1], dummy[:, 1:2], AF.Exp)

        # weights: 2-way split (first 2 k-tiles, then the rest) so the first
        # q matmul starts after ~128KB lands. NOT per-kt: every dma_start on
        # the sync queue costs ~610ns of SWDGE descriptor-gen, so 17 triggers
        # would serialize for >10us.
        nc.sync.dma_start(
            wqk_sb[:, 0:2, :],
            wqk_d[0:256, :].rearrange("(kt p) f -> p kt f", p=128),
        )
        nc.sync.dma_start(bqk_sb[:], bqk_d.rearrange("(t p) -> p t", p=128))
        # bv16 on the sync ring: on the gpsimd ring it would queue behind
        # ~4MB of x-chunk data and stall the v-bias broadcast for ~15us
        nc.sync.dma_start(bv16[:], bv_d[None, :])
        nc.sync.dma_start(
            wqk_sb[:, 2:KT, :],
            wqk_d[256:D, :].rearrange("(kt p) f -> p kt f", p=128),
        )
        nc.sync.dma_start(wv_sb[:], wv_d.rearrange("(kt p) f -> p kt f", p=128))

        # mask consts built on gpsimd AFTER the first x chunk's DMA triggers
        # are queued (emitted in the loop below); they're only needed once
        # the first diagonal attention step runs, ~15us later.
        def emit_mask_consts():
            # causal mask weight (lhsT): wm[j, k] = -240 where k > j else 0
            nc.gpsimd.memset(wm[:], -240.0)
            nc.gpsimd.affine_select(
                wm[:], wm[:], pattern=[[1, 128]], base=-1,
                channel_multiplier=-1, compare_op=OP.is_ge, fill=0.0,
            )
            # identity moving tile: ident[j, k] = (j == k)
            nc.gpsimd.memset(ident[:], 1.0)
            nc.gpsimd.affine_select(
                ident[:], ident[:], pattern=[[1, 128]], base=0,
                channel_multiplier=-1, compare_op=OP.is_ge, fill=0.0,
            )
            nc.gpsimd.affine_select(
                ident[:], ident[:], pattern=[[-1, 128]], base=0,
                channel_multiplier=1, compare_op=OP.is_ge, fill=0.0,
            )

        with ExitStack() as ctx:
            ch_pool = ctx.enter_context(tc.tile_pool(name="xchunk", bufs=4))
            sh_ps = ctx.enter_context(tc.tile_pool(name="sh_ps", bufs=2, space="PSUM"))
            sc_ps = ctx.enter_context(tc.tile_pool(name="sc_ps", bufs=2, space="PSUM"))
            attn_ps = ctx.enter_context(tc.tile_pool(name="attn_ps", bufs=1, space="PSUM"))
            sums_ps = ctx.enter_context(tc.tile_pool(name="sums_ps", bufs=1, space="PSUM"))
            epool = ctx.enter_context(tc.tile_pool(name="epool", bufs=3))
            upool = ctx.enter_context(tc.tile_pool(name="upool", bufs=2))
            rpool = ctx.enter_context(tc.tile_pool(name="rpool", bufs=2))
            opool = ctx.enter_context(tc.tile_pool(name="opool", bufs=4))

            chunks = {}

            def emit_chunk_dma(sb, split=False):
                # gpsimd SWDGE queue. Only the startup-critical first chunk is
                # split (2-way: the q chain starts after its first 256KB);
                # later chunks use one trigger (descriptor gen is ~630ns of
                # gpsimd time per dma_start).
                chunk = ch_pool.tile([128, KT, 512], bf16, tag="x", name="chunk")
                cols = slice(sb * 512, (sb + 1) * 512)
                if split:
                    nc.gpsimd.dma_start(
                        chunk[:, 0:2, :],
                        x_d[0:256, cols].rearrange("(kt p) s -> p kt s", p=128),
                    )
                    nc.gpsimd.dma_start(
                        chunk[:, 2:KT, :],
                        x_d[256:D, cols].rearrange("(kt p) s -> p kt s", p=128),
                    )
                else:
                    nc.gpsimd.dma_start(
                        chunk[:],
                        x_d[:, cols].rearrange("(kt p) s -> p kt s", p=128),
                    )
                chunks[sb] = chunk

            # first chunks' triggers go ahead of everything else on gpsimd:
            # the qkv phase consumes one 1MB chunk per ~5us and the DMA ramp
            # is the startup critical path (ch_pool bufs=4 keeps chunks 0-3
            # alive simultaneously)
            # progressive chunk triggers: front-loading all 4MB of x chunks
            # saturates the 16 shared DMA engines and starves the weight
            # DMAs on the sync ring (wv would arrive ~15us late)
            emit_chunk_dma(0, split=True)
            emit_chunk_dma(1)
            emit_mask_consts()

            # ~24 junk matmuls on const tiles keep the PE busy from ~7.5us
            # while the startup DMAs land: the HAM clock gate un-throttles
            # (4/8 -> 8/8) after ~3.4us of sustained activity, so the real
            # qkv chains start warm. Rotate the output region: back-to-back
            # writes to the SAME psum region serialize on the drain.
            ps_junk = sh_ps.tile([128, 512], f32, tag="sh", name="ps_junk")
            for j in range(24):
                sl4 = slice((j % 4) * 128, (j % 4) * 128 + 128)
                nc.tensor.matmul(
                    ps_junk[:, sl4], ones_row[:], ones_row[:],
                    start=True, stop=True, skip_group_check=True,
                )

            def emit_qk(sb, ft, scalar_drain):
                dest = (qT, kTt)[ft]
                ps = sh_ps.tile([128, 512], f32, tag="sh", name="ps_qk")
                for kt in range(KT):
                    nc.tensor.matmul(
                        ps[:],
                        wqk_sb[:, kt, ft * F : (ft + 1) * F],
                        chunks[sb][:, kt, :],
                        start=(kt == 0),
                        stop=(kt == KT - 1),
                    )
                dsl = dest[:, sb * 512 : (sb + 1) * 512]
                if scalar_drain:
                    nc.scalar.activation(
                        dsl, ps[:], AF.Identity, bias=bqk_sb[:, ft : ft + 1]
                    )
                else:
                    nc.vector.tensor_scalar(
                        dsl, ps[:], bqk_sb[:, ft : ft + 1], None, OP.add
                    )

            def emit_v(sb):
                ps = sh_ps.tile([128, 512], f32, tag="sh", name="ps_v")
                for j in range(4):
                    for kt in range(KT):
                        nc.tensor.matmul(
                            ps[:, j * 128 : (j + 1) * 128],
                            chunks[sb][:, kt, j * 128 : (j + 1) * 128],
                            wv_sb[:, kt, :],
                            start=(kt == 0),
                            stop=(kt == KT - 1),
                            skip_group_check=True,
                        )
                nc.vector.scalar_tensor_tensor(
                    v_s[:, sb * 4 : (sb + 1) * 4, :], ps[:], 1.0, bias_v_bc[:],
                    OP.mult, OP.add,
                )
                chunks.pop(sb)

            def emit_qkv_block(sb):
                # trigger the chunk DMA two blocks ahead
                if sb + 2 <= 3:
                    emit_chunk_dma(sb + 2)
                # qk drains on ScalarE only for sb0 (pre-attention; after
                # that ScalarE is on the exp critical chain), DVE otherwise
                emit_qk(sb, 0, sb == 0)
                emit_qk(sb, 1, sb == 0)
                if sb == 0:
                    # v bias broadcast tile [128,512] (rows all = tiled b_v),
                    # behind sb0's qk chains so the bv16 DMA is off the
                    # startup critical path (first needed by emit_v's drain)
                    ps_bv = sh_ps.tile([128, 512], f32, tag="sh")
                    nc.tensor.matmul(
                        ps_bv[:], ones_row[:], bv16[:], start=True, stop=True
                    )
                    nc.vector.tensor_copy(bias_v_bc[:], ps_bv[:])
                emit_v(sb)
                if sb == 0:
                    # c_proj weights are first needed ~step 7; keep their DMA
                    # out of the startup critical path
                    nc.sync.dma_start(wtil_sb[:], wtil_d)

            # b=0's qkv is emitted just-in-time before the attention block
            # that first needs it: the attention steps fill the chunk-DMA
            # wait windows that used to stall the PE in the serial qkv phase
            pre_qkv = {0: 0, 4: 1, 12: 2, 24: 3}
            # b=1's qkv (sb4-7) injected between b=0's late attention steps
            inject = {}
            for n, sb in enumerate(range(4, 8)):
                base = 24 + 6 * n
                inject.setdefault(base, []).append(("dma", sb))
                inject.setdefault(base + 2, []).append(("q", sb))
                inject.setdefault(base + 4, []).append(("k", sb))
                inject.setdefault(base + 6, []).append(("v", sb))

            # ---- attention steps ----
            # b=1 ends with its qt=0 block so the final block's tail is short
            steps = []
            for b, qts in ((0, (0, 1, 2, 3)), (1, (1, 2, 3, 0))):
                for qt in qts:
                    nkj = 4 * (qt + 1)
                    for kj in range(nkj):
                        steps.append((b, qt, kj, kj == 0, kj == nkj - 1))

            state = {}
            pend_cproj = []

            def emit_scores(i):
                b, qt, kj, first, last = steps[i]
                p = kj - 4 * qt
                lo = 128 * max(p, 0)
                qi = b * S + qt * 512
                kjc = b * S + kj * 128
                psc = sc_ps.tile([128, 1024], f32, tag="sc", name="psc")
                nc.tensor.matmul(
                    psc[:, lo:512], kTt[0:64, kjc : kjc + 128],
                    qT[0:64, qi + lo : qi + 512],
                    start=True, stop=True, tile_position=(0, 0),
                )
                nc.tensor.matmul(
                    psc[:, 512 + lo : 1024], kTt[64:128, kjc : kjc + 128],
                    qT[64:128, qi + lo : qi + 512],
                    start=True, stop=True, tile_position=(64, 0),
                )
                if p >= 0:
                    # accumulate -240 onto the masked (key > query) entries of
                    # the diagonal 128x128 block of each head; exp then
                    # flushes them to ~1e-14 on its own
                    for off in (0, 512):
                        nc.tensor.matmul(
                            psc[:, off + 128 * p : off + 128 * (p + 1)],
                            wm[:], ident[:],
                            start=False, stop=True, skip_group_check=True,
                        )
                e = epool.tile([128, 1024], bf16, tag="e", name="e")
                if p > 0:
                    # columns [0:128p] are fully-masked queries for this kj:
                    # the attn/sums matmuls skip them instead of zero-filling
                    nc.scalar.activation(
                        e[:, 128 * p : 512], psc[:, 128 * p : 512], AF.Exp, scale=0.125
                    )
                    nc.scalar.activation(
                        e[:, 512 + 128 * p : 1024], psc[:, 512 + 128 * p : 1024],
                        AF.Exp, scale=0.125,
                    )
                else:
                    nc.scalar.activation(e[:], psc[:], AF.Exp, scale=0.125)
                state[i] = e

            def emit_attn(i):
                b, qt, kj, first, last = steps[i]
                p = kj - 4 * qt
                lo = 128 * max(p, 0)
                e = state.pop(i)
                if first:
                    state["attn"] = attn_ps.tile([128, 512], f32, tag="attn", name="ps_attn")
                    state["sums"] = sums_ps.tile([128, 512], f32, tag="sums", name="ps_sums")
                ps_attn, ps_sums = state["attn"], state["sums"]
                vs = v_s[:, b * 16 + kj, :]
                eA, eB = e[:, lo:512], e[:, 512 + lo : 1024]
                nc.tensor.matmul(
                    ps_attn[0:64, lo:512], vs[:, 0:64], eA,
                    start=first, stop=last, tile_position=(0, 0),
                    skip_group_check=True,
                )
                nc.tensor.matmul(
                    ps_attn[64:128, lo:512], vs[:, 64:128], eB,
                    start=first, stop=last, tile_position=(0, 64),
                    skip_group_check=True,
                )
                nc.tensor.matmul(
                    ps_sums[0:64, lo:512], ones_w[:, 0:64], eA,
                    start=first, stop=last, tile_position=(0, 0),
                    skip_group_check=True,
                )
                nc.tensor.matmul(
                    ps_sums[64:128, lo:512], ones_w[:, 0:64], eB,
                    start=first, stop=last, tile_position=(0, 64),
                    skip_group_check=True,
                )
                if last:
                    blk = b * 4 + qt
                    ps_attn = state.pop("attn")
                    ps_sums = state.pop("sums")
                    # drain the attn bank fast (split across DVE+ScalarE so
                    # it frees in ~0.4us; short holds -> no stall for the
                    # next block's accumulations); the sums bank is released
                    # by reciprocal_approx_fast reading it directly
                    unA = upool.tile([128, 512], bf16, tag="u", name="unA")
                    nc.vector.tensor_copy(unA[:], ps_attn[:])
                    rec = rpool.tile([128, 512], f32, tag="r", name="rec")
                    with nc.allow_low_precision(reason="softmax recip approx"):
                        nc.vector.reciprocal_approx_fast(rec[:], ps_sums[:])
                        nc.vector.tensor_tensor(cpr[blk][:], unA[:], rec[:], OP.mult)
                    for dt in range(0, 8, 2):
                        pend_cproj.append((blk, dt, i + 4))

            qflip = [0]

            def emit_cproj(i, limit=1, tail=False):
                # emits PAIRS of c_proj matmuls (dt, dt+1): the second MM's
                # LDWEIGHTS hides under the first's stream, the first PSUM
                # drain goes to DVE and the second rotates ScalarE/DVE
                # (gpsimd has no PSUM port), and both output tiles leave in
                # ONE dma trigger (~640ns of SWDGE seq time each) on the
                # sync ring (the gpsimd ring moves data ~4x slower per
                # descriptor).
                n = 0
                while pend_cproj and pend_cproj[0][2] <= i and n < limit:
                    blk, dt, _ = pend_cproj.pop(0)
                    ot = opool.tile([128, 2, 512], bf16, tag="ot", name="ot")
                    for s in range(2):
                        pcp = sh_ps.tile([128, 512], f32, tag="sh", name="pcp")
                        nc.tensor.matmul(
                            pcp[:], wtil_sb[:, (dt + s) * 128 : (dt + s + 1) * 128],
                            cpr[blk][:], start=True, stop=True,
                        )
                        if s == 0:
                            nc.vector.tensor_copy(ot[:, 0, :], pcp[:])
                        elif qflip[0] % 2:
                            nc.scalar.activation(ot[:, 1, :], pcp[:], AF.Identity)
                        else:
                            nc.vector.tensor_copy(ot[:, 1, :], pcp[:])
                    qflip[0] += 1
                    nc.sync.dma_start(
                        out_d[dt : dt + 2, :, blk * 512 : (blk + 1) * 512].rearrange(
                            "t p s -> p t s"
                        ),
                        ot[:],
                    )
                    n += 1

            for i in range(len(steps)):
                if i in pre_qkv:
                    emit_qkv_block(pre_qkv[i])
                emit_scores(i)
                for item in inject.pop(i, []):
                    kind, sb = item
                    if kind == "dma":
                        emit_chunk_dma(sb)
                    elif kind == "q":
                        emit_qk(sb, 0, False)
                    elif kind == "k":
                        emit_qk(sb, 1, False)
                    else:
                        emit_v(sb)
                # cproj BEFORE attn: the attn pair depends on exp(i-1) which
                # often has a few hundred ns left when scores(i) finishes;
                # the (independent) cproj pair fills that window instead of
                # the PE stalling head-of-line on the attn matmul.
                emit_cproj(i, limit=1)
                if i > 0:
                    emit_attn(i - 1)
            emit_attn(len(steps) - 1)
            emit_cproj(10**9, limit=10**9, tail=True)

        for free in reversed(frees):
            free()


class TileCtx:
    """Thin helper so _build can use `tc.tile` / `tc.tile_pool` uniformly."""

    def __init__(self, tile_mod, nc):
        self._tc = tile_mod.TileContext(nc)

    def __enter__(self):
        self._tc.__enter__()
        return self._tc

    def __exit__(self, *exc):
        return self._tc.__exit__(*exc)


def _shard_inputs(inputs):
    import ml_dtypes

    bf = ml_dtypes.bfloat16
    xT = np.ascontiguousarray(
        np.asarray(inputs["hidden_states"], dtype=np.float32).reshape(BS, D).T
    ).astype(bf)
    Wa = np.asarray(inputs["c_attn_w"], dtype=np.float32)
    ba = np.asarray(inputs["c_attn_b"], dtype=np.float32)
    Wp = np.asarray(inputs["c_proj_w"], dtype=np.float32)
    proj = np.asarray(inputs["projectors"], dtype=np.float32)

    in_maps = []
    F = HPC * HD
    for c in range(N_CORES):
        sl = slice(c * F, (c + 1) * F)
        wtil = np.einsum(
            "hde,hef->hdf",
            proj[HPC * c : HPC * (c + 1)],
            Wp[sl, :].reshape(HPC, HD, D),
        ).reshape(F, D)
        in_maps.append(
            {
                "xT": xT,
                "w_qk": np.ascontiguousarray(
                    np.concatenate(
                        [Wa[:, sl], Wa[:, D + c * F : D + (c + 1) * F]], axis=1
                    )
                ).astype(bf),
                "w_v": np.ascontiguousarray(
                    Wa[:, 2 * D + c * F : 2 * D + (c + 1) * F]
                ).astype(bf),
                "b_qk": np.ascontiguousarray(
                    np.concatenate([ba[sl], ba[D + c * F : D + (c + 1) * F]])
                ),
                "b_v512": np.ascontiguousarray(
                    np.tile(ba[2 * D + c * F : 2 * D + (c + 1) * F], 4)
                ).astype(bf),
                "w_til": np.ascontiguousarray(wtil).astype(bf),
            }
        )
    return in_maps


def _get_nc():
    if "nc" not in _CACHE:
        from concourse import bacc

        nc = bacc.Bacc("TRN2", debug=False, num_devices=N_CORES)
        _build(nc)
        nc.compile()
        _CACHE["nc"] = nc
    return _CACHE["nc"]


def _run(inputs, trace=False, trace_kwargs=None):
    from concourse.bass_utils import run_bass_kernel_spmd

    nc = _get_nc()
    in_maps = _shard_inputs(inputs)
    res = run_bass_kernel_spmd(
        nc,
        in_maps,
        core_ids=list(range(N_CORES)),
        trace=trace,
        **(trace_kwargs or {}),
    )
    acc = np.zeros((8, 128, BS), dtype=np.float32)
    for r in res.results:
        acc += np.asarray(r["outT"], dtype=np.float32)
    bp = np.asarray(inputs["c_proj_b"], dtype=np.float32)
    out = acc.transpose(2, 0, 1).reshape(BS, D) + bp[None, :]
    return np.ascontiguousarray(out.reshape(B, S, D)), res


def kernel(**inputs) -> np.ndarray:
    out, _ = _run(inputs, trace=False)
    return out


def simulate_core(inputs, core=0):
    """CoreSim one core's program (for correctness debugging). Returns outT."""
    from concourse.bass_interp import CoreSim

    nc = _get_nc()
    in_maps = _shard_inputs(inputs)
    sim = CoreSim(nc, trace=False)
    for name, arr in in_maps[core].items():
        sim.tensor(name)[:] = arr
    sim.simulate()
    return np.array(sim.tensor("outT"))


# revision 42
# speedup vs baseline: 1.0198x; 1.0198x over previous
# Trainium2 Bass kernel for nn_CompressedGPT2Attention.
#
# Model: B=2, S=2048, D=1024, H=16 heads of HD=64.
#   qkv = x @ c_attn_w + c_attn_b ; causal attention per head;
#   per-head symmetric projector on the attention output; out = attn @ c_proj_w + b.
#
# Sharding (megatron-style tensor parallel over heads, 8 cores x 2 heads):
#   - every core gets the full hidden_states
#   - c_attn (q,k,v) columns sharded by head; the per-head projector is folded
#     into c_proj on the host (W~_h = proj_h @ c_proj_rows_h)
#   - each core writes a full-shape bf16 partial; host does the f32 all-reduce
#     + bias.
#
# v4 changes (vs the 190us v3):
#   - softmax reciprocal via the single-instruction DVE reciprocal_approx_fast
#     (reads the sums PSUM bank directly) instead of the 3.3us iterative
#     RECIPROCAL / the ScalarE ln+exp trick. ScalarE now only ever runs
#     Exp/Identity, which share one ACT table set -> no mid-kernel
#     ACT_TABLE_LOADs (was 9us of them).
#   - causal mask applied by accumulating a constant -240 strict-lower
#     triangular matrix into the scores PSUM with one extra PE matmul
#     (128-col identity stream) instead of gpsimd affine_select on e after
#     exp: removes the exp->gpsimd->attn cross-engine hop on every diagonal
#     step, and exp(score-240*mask) underflows to 0 on its own.
#   - startup weight DMAs and the first x chunk are split per 128-row k-tile
#     so the first qkv matmul only waits for the first 64-128KB slice, not the
#     full tensor (later chunks stay single-trigger: SWDGE descriptor gen is
#     ~630ns per 2D dma_start on the gpsimd queue).
#   - ~30 junk 128-col matmuls on const tiles warm the PE HAM clock gate
#     (4/8 -> 8/8 takes ~3.4us of sustained busy) while the startup DMAs are
#     still in flight, so qkv starts at 2.4GHz instead of 1.2.
#   - the final block's output DMAs alternate between the sync and gpsimd
#     queues so the 1MB tail drain uses two rings.

import numpy as np

B, S, D, H, HD = 2, 2048, 1024, 16, 64
BS = B * S
N_CORES = 8
HPC = H // N_CORES  # heads per core = 2

_CACHE = {}


def _build(nc):
    import concourse.bass as bass
    import concourse.mybir as mybir
    import concourse.tile as tile
    from contextlib import ExitStack

    f32 = mybir.dt.float32
    bf16 = mybir.dt.bfloat16
    AF = mybir.ActivationFunctionType
    OP = mybir.AluOpType

    x_d = nc.dram_tensor("xT", [D, BS], bf16, kind="ExternalInput").ap()
    wqk_d = nc.dram_tensor("w_qk", [D, 2 * HPC * HD], bf16, kind="ExternalInput").ap()
    wv_d = nc.dram_tensor("w_v", [D, HPC * HD], bf16, kind="ExternalInput").ap()
    bqk_d = nc.dram_tensor("b_qk", [2 * HPC * HD], f32, kind="ExternalInput").ap()
    bv_d = nc.dram_tensor("b_v512", [512], bf16, kind="ExternalInput").ap()
    wtil_d = nc.dram_tensor("w_til", [HPC * HD, D], bf16, kind="ExternalInput").ap()
    out_d = nc.dram_tensor("outT", [8, 128, BS], bf16, kind="ExternalOutput").ap()

    F = HPC * HD  # 128 features (2 heads stacked)
    NB = BS // 512
    KT = D // 128

    with TileCtx(tile, nc) as tc:
        frees = []

        def ptile(shape, dtype, name):
            t, free = tc.tile(shape, dtype, name=name)
            frees.append(free)
            return t

        qT = ptile([128, BS], bf16, "qT")
        kTt = ptile([128, BS], bf16, "kTt")
        v_s = ptile([128, BS // 128, 128], bf16, "v_s")
        wqk_sb = ptile([128, KT, 2 * F], bf16, "wqk_sb")
        wv_sb = ptile([128, KT, F], bf16, "wv_sb")
        wtil_sb = ptile([128, D], bf16, "wtil_sb")
        bqk_sb = ptile([128, 2], f32, "bqk_sb")
        ones_w = ptile([128, 64], bf16, "ones_w")
        ones_row = ptile([1, 128], bf16, "ones_row")
        bv16 = ptile([1, 512], bf16, "bv16")
        bias_v_bc = ptile([128, 512], f32, "bias_v_bc")
        dummy = ptile([1, 2], f32, "dummy")
        wm = ptile([128, 128], bf16, "wm")  # causal-mask weight: -240 where k>j
        ident = ptile([128, 128], bf16, "ident")
        cpr = [ptile([128, 512], bf16, f"cpr{i}") for i in range(NB)]

        nc.vector.memset(ones_w[:], 1.0)
        nc.vector.memset(ones_row[:], 1.0)
        nc.vector.memset(dummy[:], 1.0)
        # preload the exp ACT table set while the PE ramps on qkv; Identity
        # shares the same set so this is the only table load in the kernel
        nc.scalar.activation(dummy[:, 0:1], dummy[:, 1:2], AF.Exp)

        # weights: 2-way split (first 2 k-tiles, then the rest) so the first
        # q matmul starts after ~128KB lands. NOT per-kt: every dma_start on
        # the sync queue costs ~610ns of SWDGE descriptor-gen, so 17 triggers
        # would serialize for >10us.
        nc.sync.dma_start(
            wqk_sb[:, 0:2, :],
            wqk_d[0:256, :].rearrange("(kt p) f -> p kt f", p=128),
        )
        nc.sync.dma_start(bqk_sb[:], bqk_d.rearrange("(t p) -> p t", p=128))
        # bv16 on the sync ring: on the gpsimd ring it would queue behind
        # ~4MB of x-chunk data and stall the v-bias broadcast for ~15us
        nc.sync.dma_start(bv16[:], bv_d[None, :])
        nc.sync.dma_start(
            wqk_sb[:, 2:KT, :],
            wqk_d[256:D, :].rearrange("(kt p) f -> p kt f", p=128),
        )
        nc.sync.dma_start(wv_sb[:], wv_d.rearrange("(kt p) f -> p kt f", p=128))

        # mask consts built on gpsimd AFTER the first x chunk's DMA triggers
        # are queued (emitted in the loop below); they're only needed once
        # the first diagonal attention step runs, ~15us later.
        def emit_mask_consts():
            # causal mask weight (lhsT): wm[j, k] = -240 where k > j else 0
            nc.gpsimd.memset(wm[:], -240.0)
            nc.gpsimd.affine_select(
                wm[:], wm[:], pattern=[[1, 128]], base=-1,
                channel_multiplier=-1, compare_op=OP.is_ge, fill=0.0,
            )
            # identity moving tile: ident[j, k] = (j == k)
            nc.gpsimd.memset(ident[:], 1.0)
            nc.gpsimd.affine_select(
                ident[:], ident[:], pattern=[[1, 128]], base=0,
                channel_multiplier=-1, compare_op=OP.is_ge, fill=0.0,
            )
            nc.gpsimd.affine_select(
                ident[:], ident[:], pattern=[[-1, 128]], base=0,
                channel_multiplier=1, compare_op=OP.is_ge, fill=0.0,
            )

        with ExitStack() as ctx:
            ch_pool = ctx.enter_context(tc.tile_pool(name="xchunk", bufs=4))
            sh_ps = ctx.enter_context(tc.tile_pool(name="sh_ps", bufs=2, space="PSUM"))
            sc_ps = ctx.enter_context(tc.tile_pool(name="sc_ps", bufs=2, space="PSUM"))
            attn_ps = ctx.enter_context(tc.tile_pool(name="attn_ps", bufs=1, space="PSUM"))
            sums_ps = ctx.enter_context(tc.tile_pool(name="sums_ps", bufs=1, space="PSUM"))
            epool = ctx.enter_context(tc.tile_pool(name="epool", bufs=3))
            upool = ctx.enter_context(tc.tile_pool(name="upool", bufs=2))
            rpool = ctx.enter_context(tc.tile_pool(name="rpool", bufs=2))
            opool = ctx.enter_context(tc.tile_pool(name="opool", bufs=4))

            chunks = {}

            def emit_chunk_dma(sb, split=False):
                # gpsimd SWDGE queue. Only the startup-critical first chunk is
                # split (2-way: the q chain starts after its first 256KB);
                # later chunks use one trigger (descriptor gen is ~630ns of
                # gpsimd time per dma_start).
                chunk = ch_pool.tile([128, KT, 512], bf16, tag="x", name="chunk")
                cols = slice(sb * 512, (sb + 1) * 512)
                if split:
                    nc.gpsimd.dma_start(
                        chunk[:, 0:2, :],
                        x_d[0:256, cols].rearrange("(kt p) s -> p kt s", p=128),
                    )
                    nc.gpsimd.dma_start(
                        chunk[:, 2:KT, :],
                        x_d[256:D, cols].rearrange("(kt p) s -> p kt s", p=128),
                    )
                else:
                    nc.gpsimd.dma_start(
                        chunk[:],
                        x_d[:, cols].rearrange("(kt p) s -> p kt s", p=128),
                    )
                chunks[sb] = chunk

            # first chunks' triggers go ahead of everything else on gpsimd:
            # the qkv phase consumes one 1MB chunk per ~5us and the DMA ramp
            # is the startup critical path (ch_pool bufs=4 keeps chunks 0-3
            # alive simultaneously)
            # progressive chunk triggers: front-loading all 4MB of x chunks
            # saturates the 16 shared DMA engines and starves the weight
            # DMAs on the sync ring (wv would arrive ~15us late)
            emit_chunk_dma(0, split=True)
            emit_chunk_dma(1)
            emit_chunk_dma(2)
            emit_mask_consts()

            # ~24 junk matmuls on const tiles keep the PE busy from ~7.5us
            # while the startup DMAs land: the HAM clock gate un-throttles
            # (4/8 -> 8/8) after ~3.4us of sustained activity, so the real
            # qkv chains start warm. Rotate the output region: back-to-back
            # writes to the SAME psum region serialize on the drain.
            ps_junk = sh_ps.tile([128, 512], f32, tag="sh", name="ps_junk")
            for j in range(24):
                sl4 = slice((j % 4) * 128, (j % 4) * 128 + 128)
                nc.tensor.matmul(
                    ps_junk[:, sl4], ones_row[:], ones_row[:],
                    start=True, stop=True, skip_group_check=True,
                )

            def emit_qk(sb, ft, scalar_drain):
                dest = (qT, kTt)[ft]
                ps = sh_ps.tile([128, 512], f32, tag="sh", name="ps_qk")
                for kt in range(KT):
                    nc.tensor.matmul(
                        ps[:],
                        wqk_sb[:, kt, ft * F : (ft + 1) * F],
                        chunks[sb][:, kt, :],
                        start=(kt == 0),
                        stop=(kt == KT - 1),
                    )
                dsl = dest[:, sb * 512 : (sb + 1) * 512]
                if scalar_drain:
                    nc.scalar.activation(
                        dsl, ps[:], AF.Identity, bias=bqk_sb[:, ft : ft + 1]
                    )
                else:
                    nc.vector.tensor_scalar(
                        dsl, ps[:], bqk_sb[:, ft : ft + 1], None, OP.add
                    )

            def emit_v(sb):
                ps = sh_ps.tile([128, 512], f32, tag="sh", name="ps_v")
                for j in range(4):
                    for kt in range(KT):
                        nc.tensor.matmul(
                            ps[:, j * 128 : (j + 1) * 128],
                            chunks[sb][:, kt, j * 128 : (j + 1) * 128],
                            wv_sb[:, kt, :],
                            start=(kt == 0),
                            stop=(kt == KT - 1),
                            skip_group_check=True,
                        )
                nc.vector.scalar_tensor_tensor(
                    v_s[:, sb * 4 : (sb + 1) * 4, :], ps[:], 1.0, bias_v_bc[:],
                    OP.mult, OP.add,
                )
                chunks.pop(sb)

            # ---- sb0-3 upfront (b=0 data); ScalarE does the qk drains ----
            for sb in range(4):
                if sb == 3:
                    emit_chunk_dma(sb)
                emit_qk(sb, 0, True)
                emit_qk(sb, 1, True)
                if sb == 0:
                    # v bias broadcast tile [128,512] (rows all = tiled b_v),
                    # behind sb0's qk chains so the bv16 DMA is off the
                    # startup critical path (first needed by emit_v's drain)
                    ps_bv = sh_ps.tile([128, 512], f32, tag="sh")
                    nc.tensor.matmul(
                        ps_bv[:], ones_row[:], bv16[:], start=True, stop=True
                    )
                    nc.vector.tensor_copy(bias_v_bc[:], ps_bv[:])
                emit_v(sb)
            # c_proj weights are first needed ~step 5; keep their DMA out of
            # the startup critical path
            nc.sync.dma_start(wtil_sb[:], wtil_d)

            # remaining qkv work (b=1), injected between early attention steps
            inject = {}
            for n, sb in enumerate(range(4, 8)):
                base = 6 * n
                inject.setdefault(base, []).append(("dma", sb))
                inject.setdefault(base + 2, []).append(("q", sb))
                inject.setdefault(base + 4, []).append(("k", sb))
                inject.setdefault(base + 6, []).append(("v", sb))

            # ---- attention steps ----
            # b=1 ends with its qt=0 block so the final block's tail is short
            steps = []
            for b, qts in ((0, (0, 1, 2, 3)), (1, (1, 2, 3, 0))):
                for qt in qts:
                    nkj = 4 * (qt + 1)
                    for kj in range(nkj):
                        steps.append((b, qt, kj, kj == 0, kj == nkj - 1))

            state = {}
            pend_cproj = []

            def emit_scores(i):
                b, qt, kj, first, last = steps[i]
                p = kj - 4 * qt
                lo = 128 * max(p, 0)
                qi = b * S + qt * 512
                kjc = b * S + kj * 128
                psc = sc_ps.tile([128, 1024], f32, tag="sc", name="psc")
                nc.tensor.matmul(
                    psc[:, lo:512], kTt[0:64, kjc : kjc + 128],
                    qT[0:64, qi + lo : qi + 512],
                    start=True, stop=True, tile_position=(0, 0),
                )
                nc.tensor.matmul(
                    psc[:, 512 + lo : 1024], kTt[64:128, kjc : kjc + 128],
                    qT[64:128, qi + lo : qi + 512],
                    start=True, stop=True, tile_position=(64, 0),
                )
                if p >= 0:
                    # accumulate -240 onto the masked (key > query) entries of
                    # the diagonal 128x128 block of each head; exp then
                    # flushes them to ~1e-14 on its own
                    for off in (0, 512):
                        nc.tensor.matmul(
                            psc[:, off + 128 * p : off + 128 * (p + 1)],
                            wm[:], ident[:],
                            start=False, stop=True, skip_group_check=True,
                        )
                e = epool.tile([128, 1024], bf16, tag="e", name="e")
                if p > 0:
                    # columns [0:128p] are fully-masked queries for this kj:
                    # the attn/sums matmuls skip them instead of zero-filling
                    nc.scalar.activation(
                        e[:, 128 * p : 512], psc[:, 128 * p : 512], AF.Exp, scale=0.125
                    )
                    nc.scalar.activation(
                        e[:, 512 + 128 * p : 1024], psc[:, 512 + 128 * p : 1024],
                        AF.Exp, scale=0.125,
                    )
                else:
                    nc.scalar.activation(e[:], psc[:], AF.Exp, scale=0.125)
                state[i] = e

            def emit_attn(i):
                b, qt, kj, first, last = steps[i]
                p = kj - 4 * qt
                lo = 128 * max(p, 0)
                e = state.pop(i)
                if first:
                    state["attn"] = attn_ps.tile([128, 512], f32, tag="attn", name="ps_attn")
                    state["sums"] = sums_ps.tile([128, 512], f32, tag="sums", name="ps_sums")
                ps_attn, ps_sums = state["attn"], state["sums"]
                vs = v_s[:, b * 16 + kj, :]
                eA, eB = e[:, lo:512], e[:, 512 + lo : 1024]
                nc.tensor.matmul(
                    ps_attn[0:64, lo:512], vs[:, 0:64], eA,
                    start=first, stop=last, tile_position=(0, 0),
                    skip_group_check=True,
                )
                nc.tensor.matmul(
                    ps_attn[64:128, lo:512], vs[:, 64:128], eB,
                    start=first, stop=last, tile_position=(0, 64),
                    skip_group_check=True,
                )
                nc.tensor.matmul(
                    ps_sums[0:64, lo:512], ones_w[:, 0:64], eA,
                    start=first, stop=last, tile_position=(0, 0),
                    skip_group_check=True,
                )
                nc.tensor.matmul(
                    ps_sums[64:128, lo:512], ones_w[:, 0:64], eB,
                    start=first, stop=last, tile_position=(0, 64),
                    skip_group_check=True,
                )
                if last:
                    blk = b * 4 + qt
                    ps_attn = state.pop("attn")
                    ps_sums = state.pop("sums")
                    # drain the attn bank fast (split across DVE+ScalarE so
                    # it frees in ~0.4us; short holds -> no stall for the
                    # next block's accumulations); the sums bank is released
                    # by reciprocal_approx_fast reading it directly
                    unA = upool.tile([128, 512], bf16, tag="u", name="unA")
                    nc.vector.tensor_copy(unA[:], ps_attn[:])
                    rec = rpool.tile([128, 512], f32, tag="r", name="rec")
                    with nc.allow_low_precision(reason="softmax recip approx"):
                        nc.vector.reciprocal_approx_fast(rec[:], ps_sums[:])
                        nc.vector.tensor_tensor(cpr[blk][:], unA[:], rec[:], OP.mult)
                    for dt in range(0, 8, 2):
                        pend_cproj.append((blk, dt, i + 4))

            qflip = [0]

            def emit_cproj(i, limit=1, tail=False):
                # emits PAIRS of c_proj matmuls (dt, dt+1): the second MM's
                # LDWEIGHTS hides under the first's stream, the first PSUM
                # drain goes to DVE and the second rotates ScalarE/DVE
                # (gpsimd has no PSUM port), and both output tiles leave in
                # ONE dma trigger (~640ns of SWDGE seq time each) on the
                # sync ring (the gpsimd ring moves data ~4x slower per
                # descriptor).
                n = 0
                while pend_cproj and pend_cproj[0][2] <= i and n < limit:
                    blk, dt, _ = pend_cproj.pop(0)
                    ot = opool.tile([128, 2, 512], bf16, tag="ot", name="ot")
                    for s in range(2):
                        pcp = sh_ps.tile([128, 512], f32, tag="sh", name="pcp")
                        nc.tensor.matmul(
                            pcp[:], wtil_sb[:, (dt + s) * 128 : (dt + s + 1) * 128],
                            cpr[blk][:], start=True, stop=True,
                        )
                        if s == 0:
                            nc.vector.tensor_copy(ot[:, 0, :], pcp[:])
                        elif qflip[0] % 2:
                            nc.scalar.activation(ot[:, 1, :], pcp[:], AF.Identity)
                        else:
                            nc.vector.tensor_copy(ot[:, 1, :], pcp[:])
                    qflip[0] += 1
                    nc.sync.dma_start(
                        out_d[dt : dt + 2, :, blk * 512 : (blk + 1) * 512].rearrange(
                            "t p s -> p t s"
                        ),
                        ot[:],
                    )
                    n += 1

            for i in range(len(steps)):
                emit_scores(i)
                for item in inject.pop(i, []):
                    kind, sb = item
                    if kind == "dma":
                        emit_chunk_dma(sb)
                    elif kind == "q":
                        emit_qk(sb, 0, False)
                    elif kind == "k":
                        emit_qk(sb, 1, False)
                    else:
                        emit_v(sb)
                # cproj BEFORE attn: the attn pair depends on exp(i-1) which
                # often has a few hundred ns left when scores(i) finishes;
                # the (independent) cproj pair fills that window instead of
                # the PE stalling head-of-line on the attn matmul.
                emit_cproj(i, limit=1)
                if i > 0:
                    emit_attn(i - 1)
            emit_attn(len(steps) - 1)
            emit_cproj(10**9, limit=10**9, tail=True)

        for free in reversed(frees):
            free()


class TileCtx:
    """Thin helper so _build can use `tc.tile` / `tc.tile_pool` uniformly."""

    def __init__(self, tile_mod, nc):
        self._tc = tile_mod.TileContext(nc)

    def __enter__(self):
        self._tc.__enter__()
        return self._tc

    def __exit__(self, *exc):
        return self._tc.__exit__(*exc)


def _shard_inputs(inputs):
    import ml_dtypes

    bf = ml_dtypes.bfloat16
    xT = np.ascontiguousarray(
        np.asarray(inputs["hidden_states"], dtype=np.float32).reshape(BS, D).T
    ).astype(bf)
    Wa = np.asarray(inputs["c_attn_w"], dtype=np.float32)
    ba = np.asarray(inputs["c_attn_b"], dtype=np.float32)
    Wp = np.asarray(inputs["c_proj_w"], dtype=np.float32)
    proj = np.asarray(inputs["projectors"], dtype=np.float32)

    in_maps = []
    F = HPC * HD
    for c in range(N_CORES):
        sl = slice(c * F, (c + 1) * F)
        wtil = np.einsum(
            "hde,hef->hdf",
            proj[HPC * c : HPC * (c + 1)],
            Wp[sl, :].reshape(HPC, HD, D),
        ).reshape(F, D)
        in_maps.append(
            {
                "xT": xT,
                "w_qk": np.ascontiguousarray(
                    np.concatenate(
                        [Wa[:, sl], Wa[:, D + c * F : D + (c + 1) * F]], axis=1
                    )
                ).astype(bf),
                "w_v": np.ascontiguousarray(
                    Wa[:, 2 * D + c * F : 2 * D + (c + 1) * F]
                ).astype(bf),
                "b_qk": np.ascontiguousarray(
                    np.concatenate([ba[sl], ba[D + c * F : D + (c + 1) * F]])
                ),
                "b_v512": np.ascontiguousarray(
                    np.tile(ba[2 * D + c * F : 2 * D + (c + 1) * F], 4)
                ).astype(bf),
                "w_til": np.ascontiguousarray(wtil).astype(bf),
            }
        )
    return in_maps


def _get_nc():
    if "nc" not in _CACHE:
        from concourse import bacc

        nc = bacc.Bacc("TRN2", debug=False, num_devices=N_CORES)
        _build(nc)
        nc.compile()
        _CACHE["nc"] = nc
    return _CACHE["nc"]


def _run(inputs, trace=False, trace_kwargs=None):
    from concourse.bass_utils import run_bass_kernel_spmd

    nc = _get_nc()
    in_maps = _shard_inputs(inputs)
    res = run_bass_kernel_spmd(
        nc,
        in_maps,
        core_ids=list(range(N_CORES)),
        trace=trace,
        **(trace_kwargs or {}),
    )
    acc = np.zeros((8, 128, BS), dtype=np.float32)
    for r in res.results:
        acc += np.asarray(r["outT"], dtype=np.float32)
    bp = np.asarray(inputs["c_proj_b"], dtype=np.float32)
    out = acc.transpose(2, 0, 1).reshape(BS, D) + bp[None, :]
    return np.ascontiguousarray(out.reshape(B, S, D)), res


def kernel(**inputs) -> np.ndarray:
    out, _ = _run(inputs, trace=False)
    return out


def simulate_core(inputs, core=0):
    """CoreSim one core's program (for correctness debugging). Returns outT."""
    from concourse.bass_interp import CoreSim

    nc = _get_nc()
    in_maps = _shard_inputs(inputs)
    sim = CoreSim(nc, trace=False)
    for name, arr in in_maps[core].items():
        sim.tensor(name)[:] = arr
    sim.simulate()
    return np.array(sim.tensor("outT"))
